# revision 1
# baseline (speedup 1.0000x reference)
"""TRN2 Bass kernel for nn_MFILoss_38225208934871 (wrap-symmetric fp8).

loss = sum((diag(S)-1)^2) + 0.2 * sum_i [ sum_j S_off[i,j]^3 / (mean_j S_off[i,j] + 1e-6) ]
with S = t_norm @ t_norm.T, t_norm = L2-row-normalized t_prime [8192, 768].

Device strategy (8 cores, SPMD-identical program, host shards/gathers):
  - Wrap-symmetric decomposition: with 64 row-blocks of 128, each block i
    computes tiles (i, j) for j = i..i+32 (mod 64).  Every unordered pair
    (d = 1..31) is computed once: the owner row gets it via a row-reduction
    (fp32 accum), the partner row via a column-reduction (ones-matmul on an
    fp8 cube tile into PSUM).  d = 32 tiles are computed at both ends
    (row-reductions only); diagonal blocks are row-reduced only.
  - Each core owns 8 consecutive row-blocks; its moving operand is the
    host-rotated column window [1024c, 1024c + 5120) mod 8192, so all
    cores run the identical instruction stream on rotated data.
  - GEMMs in fp8e4m3 (inputs scaled x16) with DoubleRow perf mode
    (K=256 per matmul).  Cube pipeline: ScalarE square (bf16), then a
    scalar_tensor_tensor (P*c1)*sq with fp32 row-accumulate, emitting the
    scaled cube tile (fp8, or bf16 for diagonal-containing tiles where the
    S=1 diagonal would overflow fp8).  Work is greedily balanced across
    ACT/DVE/Pool at build time.
  - mean_neg is ill-conditioned; it is computed exactly on host (fp64).
    The 256 rows with smallest |mean_neg+eps| are recomputed on-device in
    fp8 hi+lo (~bf16 quality), column-sharded across cores.
  - Host assembles everything in fp64 (diag^3 subtraction, sensitive-row
    swap, final divide).

Inputs are full/unsharded; output is the full scalar loss (float32).
"""

import numpy as np
from contextlib import ExitStack

V = 8192
D = 768
NCORES = 8
B = 128                      # row/col block
RB_PER_CORE = 8              # row-blocks per core (1024 rows)
T = 33                       # staircase blocks per row-block (d = 0..32)
LOCALW = B * (RB_PER_CORE - 1) + B * T      # 5120 local columns
NMOVCH = LOCALW // 1024      # 5 input DMA chunks
NCB = LOCALW // B            # 40 local column blocks (colsum slots)
NG = 9                       # 8 x 512-wide groups + 1 x 128-wide (d=32)

S1 = 16.0                    # fp8 input scale (bulk)
S2 = 64.0                    # fp8 hi/lo input scale (refinement)
CSC = 4096.0                 # cube output scale: cb = S^3 * CSC
C1 = CSC / S1**6             # = 2^-12, exact
C1R = CSC / S2**6            # = 2^-24, exact

REFINE_K = 256
RB = REFINE_K // 128         # 2 stationary blocks of refined rows
SEGW = V // NCORES           # 1024-column refinement segment per core

EPS = 1e-6
LAMBDA = 0.2

# build-time engine cost estimates (ns) for greedy balancing.
# Pipeline options per tile (Pool/GPSIMD can only run tensor_tensor on
# SBUF operands, so its path needs ACT staging):
#   "dve": one fused cube+reduce custom-DVE op straight from PSUM;
#   "apc": ACT scaled-copy + ACT scaled-square (PSUM->SBUF bf16),
#          Pool TT multiply, ACT copy-with-accum (rowsum + final cb).
_PIPE_COST = {
    "dve": {"dve": 658.0},
    "apc": {"act": 2023.0, "pool": 1111.0},
}

_cache = {}


def _register_cube_op():
    """Register a fused cube+row-reduce custom DVE op:
        out = (in0^2 * in0) * c1 ;  accum_out = c0 + sum(out_fp32)
    Follows the documented extension flow in concourse/dve_ops.py."""
    if "cube_op" in _cache:
        return _cache["cube_op"]
    import concourse.dve_ops as dve_ops
    from concourse.dve_ops import DveOp
    from concourse.dve_spec import Spec, Src0, C0, C1, sq, lower, _has_src1
    from concourse.dve_uop import DveOpSpec
    from operator import add
    import numpy as np_

    name = "CUBE_REDUCE_MFI"

    def _ref(in0, in1, c0, c1, c2):
        b = ((in0.astype(np_.float32) ** 2) * in0.astype(np_.float32)
             * c1).astype(np_.float32)
        return b, c0 + b.reshape(b.shape[0], -1).sum(axis=-1, keepdims=True)

    spec = Spec(body=sq(Src0) * Src0 * C1, accum=add, accum_init=C0,
                reference=_ref)

    if name not in dve_ops._SUB_OPCODE_FOR_NAME:
        row = max(dve_ops._SUB_OPCODE_FOR_NAME.values()) + 1
        assert row < 0x20
        dve_ops._SUB_OPCODE_FOR_NAME[name] = row

    # pin the sha by lowering once per supported ver
    shas = {}
    for ver in ("v3", "v4"):
        try:
            s = DveOpSpec(name=name,
                          opcode=dve_ops._SUB_OPCODE_FOR_NAME[name],
                          uops=lower(spec, ver=ver),
                          rd1_en=_has_src1(spec))
            shas[ver] = s.sha(ver)
        except Exception:
            pass
    op = DveOp(name, spec, subdim=False, uops_sha=shas)
    if all(o.name != name for o in dve_ops.OPS):
        dve_ops.OPS.append(op)
    dve_ops.CUSTOM_DVE_SPECS[name] = spec
    _cache["cube_op"] = op
    return op


def _build():
    import concourse.tile as tile
    from concourse import bacc, mybir

    F32 = mybir.dt.float32
    BF16 = mybir.dt.bfloat16
    FP8 = mybir.dt.float8e4
    MULT = mybir.AluOpType.mult
    DR = mybir.MatmulPerfMode.DoubleRow
    SQUARE = mybir.ActivationFunctionType.Square

    nc = bacc.Bacc("TRN2", target_bir_lowering=False, debug=False,
                   num_devices=NCORES)

    d_mov = nc.dram_tensor("mov8", [128, 3, 2, LOCALW - 512], FP8,
                           kind="ExternalInput").ap()
    d_sta = nc.dram_tensor("sta8", [128, 3, 2, 1024], FP8,
                           kind="ExternalInput").ap()
    d_shi = nc.dram_tensor("senshi", [128, 3, 2, REFINE_K], FP8,
                           kind="ExternalInput").ap()
    d_slo = nc.dram_tensor("senslo", [128, 3, 2, REFINE_K], FP8,
                           kind="ExternalInput").ap()
    d_ghi = nc.dram_tensor("seghi", [128, 3, 2, SEGW], FP8,
                           kind="ExternalInput").ap()
    d_glo = nc.dram_tensor("seglo", [128, 3, 2, SEGW], FP8,
                           kind="ExternalInput").ap()
    NSLOT = RB_PER_CORE * NG + RB * 2          # 72 main + 4 refinement
    d_rc = nc.dram_tensor("rc", [128, NSLOT], F32,
                          kind="ExternalOutput").ap()
    d_cols = nc.dram_tensor("cols", [128, NCB], F32,
                            kind="ExternalOutput").ap()

    cube_op = _register_cube_op()

    # greedy engine balancer state
    eng_t = {"act": 15000.0, "dve": 0.0, "pool": 15000.0}

    def pick(width, allow_aap=True):
        scale = width / 512.0
        best = None
        for pname, costs in _PIPE_COST.items():
            if pname == "aap" and not allow_aap:
                continue
            t = dict(eng_t)
            for e, c in costs.items():
                t[e] += c * scale
            m = max(t.values())
            if best is None or m < best[0]:
                best = (m, pname)
        pname = best[1]
        for e, c in _PIPE_COST[pname].items():
            eng_t[e] += c * scale
        return pname

    with tile.TileContext(nc) as tc, ExitStack() as ctx:
        in_pool = ctx.enter_context(tc.tile_pool(name="inp", bufs=1))
        ref_pool = ctx.enter_context(tc.tile_pool(name="refin", bufs=1))
        psum_pool = ctx.enter_context(tc.tile_pool(name="ps", bufs=4,
                                                   space="PSUM"))
        psum_a = ctx.enter_context(tc.tile_pool(name="psa", bufs=3,
                                                space="PSUM"))
        col_pool = ctx.enter_context(tc.tile_pool(name="cs", bufs=1,
                                                  space="PSUM"))
        sq_pool = ctx.enter_context(tc.tile_pool(name="sq", bufs=16))
        cb_pool = ctx.enter_context(tc.tile_pool(name="cb", bufs=1))
        rc_pool = ctx.enter_context(tc.tile_pool(name="rc", bufs=1))

        # ---- inputs ----
        # local cols [0, 1024) duplicate the stationary (own rows), so the
        # moving buffer only covers [512, 5120); the first tiles of the
        # g=0 row run on sta8 alone while mov streams in.
        sta8 = in_pool.tile([128, 3, 2, 1024], FP8, tag="sta8")
        nc.sync.dma_start(sta8[:], d_sta)
        MOVW = LOCALW - 512
        mov8 = in_pool.tile([128, 3, 2, MOVW], FP8, tag="mov8")
        bounds = list(range(0, MOVW, 1024)) + [MOVW]
        pieces = list(zip(bounds[:-1], bounds[1:]))
        for a, b in pieces[:-1]:
            nc.sync.dma_start(mov8[:, :, :, a:b], d_mov[:, :, :, a:b])

        # refinement inputs squeeze in before the last mov piece (HWDGE
        # path; needed only by the late-emitted refinement tiles)
        shi = ref_pool.tile([128, 3, 2, REFINE_K], FP8, tag="shi")
        nc.sync.dma_start(shi[:], d_shi)
        slo = ref_pool.tile([128, 3, 2, REFINE_K], FP8, tag="slo")
        nc.sync.dma_start(slo[:], d_slo)
        ghi = ref_pool.tile([128, 3, 2, SEGW], FP8, tag="ghi")
        nc.sync.dma_start(ghi[:], d_ghi)
        glo = ref_pool.tile([128, 3, 2, SEGW], FP8, tag="glo")
        nc.sync.dma_start(glo[:], d_glo)

        a, b = pieces[-1]
        nc.sync.dma_start(mov8[:, :, :, a:b], d_mov[:, :, :, a:b])

        # ---- constants / buffers ----
        ones8 = cb_pool.tile([128, 1], FP8, tag="ones8")
        nc.gpsimd.memset(ones8[:], 1.0)
        ones16 = cb_pool.tile([128, 1], BF16, tag="ones16")
        nc.gpsimd.memset(ones16[:], 1.0)
        zeros8 = cb_pool.tile([128, 128], FP8, tag="zeros8")
        nc.gpsimd.memset(zeros8[:], 0.0)
        onesNB = cb_pool.tile([128, NCB], FP8, tag="onesNB")
        nc.gpsimd.memset(onesNB[:], 1.0)
        # cube-tile arena sized for the whole kernel: no reuse deps
        NB16 = NG * RB_PER_CORE + RB * 2
        cbB = cb_pool.tile([128, NB16, 512], BF16, tag="cbB")

        rc = rc_pool.tile([128, NSLOT], F32, tag="rc")
        colsb = rc_pool.tile([128, NCB], F32, tag="colsb")

        # colsum region: col b = partial column sums of local col-block b
        creg = col_pool.tile([128, NCB], F32, tag="creg")
        # zero-init (start=True writes exact zeros everywhere)
        nc.tensor.matmul(creg[:], zeros8[:], onesNB[:],
                         start=True, stop=False, skip_group_check=True)

        # delayed colsum matmuls: (cb_ap_512wide, is_fp8, [(k, b)])
        pending = []
        ring_i = [0]
        ringb_i = [0]

        def flush_pending(keep=0):
            while len(pending) > keep:
                cb_ap, is8, blocks = pending.pop(0)
                for (k, b) in blocks:
                    lhsT = cb_ap[:, 128 * k:128 * (k + 1)]
                    ones = ones8 if is8 else ones16
                    nc.tensor.matmul(creg[:, b:b + 1], lhsT, ones[:],
                                     start=False, stop=False,
                                     skip_group_check=True)

        # ---- PE warmup: keep the PE array busy (and its clock warm) while
        # the input DMAs stream in; small zero matmuls into scratch PSUM.
        for _ in range(27):
            Pd = psum_pool.tile([128, 512], F32, tag="P")
            nc.tensor.matmul(Pd[:, :128], zeros8[:], zeros8[:],
                             start=True, stop=True)

        COPY = mybir.ActivationFunctionType.Copy

        def emit_elementwise(P, w, cb_out, slot, scale, scale3, pname):
            """cube pipeline: cb_out = P^3*scale, slot = rowsum (fp32).
            scale3 = scale ** (1/3), an exact power of two."""
            if pname == "dve":
                nc.vector._custom_dve(cube_op, out=cb_out, in0=P[:, :w],
                                      s0=0.0, s1=scale, accum_out=slot)
            else:
                s16 = sq_pool.tile([128, 512], BF16, tag="s16")
                nc.scalar.mul(s16[:, :w], P[:, :w], scale3)
                sq = sq_pool.tile([128, 512], BF16, tag="sq")
                nc.scalar.activation(sq[:, :w], P[:, :w], SQUARE,
                                     scale=scale3)
                ct = sq_pool.tile([128, 512], BF16, tag="ct")
                nc.gpsimd.tensor_tensor(ct[:, :w], s16[:, :w], sq[:, :w],
                                        MULT)
                nc.scalar.activation(cb_out, ct[:, :w], COPY,
                                     accum_out=slot)
            return pname

        ref_tiles = [(rb, qq) for rb in range(RB) for qq in range(SEGW // 512)]
        ref_pairs = [(shi, ghi), (shi, glo), (slo, ghi)]

        def emit_refinement_tile(rb, qq):
            pname = pick(512, False)
            P = psum_pool.tile([128, 512], F32, tag="P")
            n_mm = len(ref_pairs) * 3
            i_mm = 0
            for (wl, wr) in ref_pairs:
                for kc2 in range(3):
                    nc.tensor.matmul(
                        P[:],
                        wl[:, kc2, :, rb * 128:(rb + 1) * 128],
                        wr[:, kc2, :, qq * 512:(qq + 1) * 512],
                        start=(i_mm == 0), stop=(i_mm == n_mm - 1),
                        perf_mode=DR)
                    i_mm += 1
            nb = ringb_i[0]
            ringb_i[0] += 1
            slot = rc[:, RB_PER_CORE * NG + rb * 2 + qq:
                      RB_PER_CORE * NG + rb * 2 + qq + 1]
            # exact-precision pipeline (ScalarE square in fp32 + DVE STT
            # accum): the refinement rows are the den-sensitive ones
            sqf = sq_pool.tile([128, 512], F32, tag="sqf")
            nc.scalar.activation(sqf[:], P[:], SQUARE)
            nc.vector.scalar_tensor_tensor(cbB[:, nb, :], P[:], C1R, sqf[:],
                                           MULT, MULT, accum_out=slot)

        def emit_main_tile(g, i):
            tid = g * RB_PER_CORE + i
            w = 512 if g < NG - 1 else 128
            c0 = 128 * i + 512 * g
            pname = pick(w, allow_aap=(g < 7 or g == NG - 1))
            pool = psum_pool if pname == "dve" else psum_a
            P = pool.tile([128, 512], F32, tag="P")
            for kc2 in range(3):
                if c0 + w <= 1024:
                    rhs = sta8[:, kc2, :, c0:c0 + w]
                else:
                    rhs = mov8[:, kc2, :, c0 - 512:c0 - 512 + w]
                nc.tensor.matmul(
                    P[:, :w],
                    sta8[:, kc2, :, 128 * i:128 * (i + 1)],
                    rhs,
                    start=(kc2 == 0), stop=(kc2 == 2), perf_mode=DR)

            slot = rc[:, tid:tid + 1]
            nb = ringb_i[0]
            ringb_i[0] += 1
            cb_out = cbB[:, nb, :w]
            cs_ap = cbB[:, nb, :]
            emit_elementwise(P, w, cb_out, slot, C1, 2.0 ** -4, pname)
            if g == 0:
                _cache.setdefault("diag_paths", {})[i] = pname

            # colsum blocks (exclude diag block for g=0; none for g=8)
            b0 = i + 4 * g
            if g == 0:
                blocks = [(k, b0 + k) for k in range(1, 4)]
            elif g < NG - 1:
                blocks = [(k, b0 + k) for k in range(4)]
            else:
                blocks = []
            if blocks:
                pending.append((cs_ap, False, blocks))
            flush_pending(keep=12)

        for g in range(NG - 1):
            for i in range(RB_PER_CORE):
                emit_main_tile(g, i)
                # spread the narrow d=32 tiles behind the g=6 row and the
                # refinement tiles behind the g=7 row
                if g == 6:
                    emit_main_tile(NG - 1, i)
                if g == 7 and i % 2 == 1 and ref_tiles:
                    emit_refinement_tile(*ref_tiles.pop(0))

        # main rowsum slots are complete before the refinement tail ends;
        # ship them early so only the 4 refinement slots drain at the end
        nc.sync.dma_start(d_rc[:, :RB_PER_CORE * NG],
                          rc[:, :RB_PER_CORE * NG])

        while ref_tiles:
            emit_refinement_tile(*ref_tiles.pop(0))
        flush_pending()

        # ---- outputs ----
        nc.scalar.copy(colsb[:], creg[:])
        nc.sync.dma_start(d_rc[:, RB_PER_CORE * NG:],
                          rc[:, RB_PER_CORE * NG:])
        nc.sync.dma_start(d_cols, colsb[:])

    nc.compile()
    return nc


def _prep(t_prime: np.ndarray):
    import ml_dtypes
    F8 = ml_dtypes.float8_e4m3fn

    t64 = t_prime.astype(np.float64)
    norm = np.maximum(np.sqrt((t64 * t64).sum(1, keepdims=True)), 1e-12)
    tn32 = (t64 / norm).astype(np.float32)           # [V, D]
    tn64 = tn32.astype(np.float64)

    # exact (fp64) mean_neg and collapse on host
    s = tn64.sum(0)
    rowsum = tn64 @ s
    diag = (tn64 * tn64).sum(1)
    mean_neg = (rowsum - diag) / (V - 1)
    den = mean_neg + EPS
    collapse = np.sum((diag - 1.0) ** 2)

    # fp8 bulk operand (scaled x16), in device layout [128, 3, 2, V]
    t8 = (tn32 * S1).astype(F8)                      # [V, D] fp8
    t8_64 = t8.astype(np.float64) / S1
    diag8 = (t8_64 * t8_64).sum(1)                   # device bulk diagonal
    # predicted device diagonal cube (pre-/CSC scale), path-dependent:
    # "dve" computes fp32 P^3*C1 from exact P; "aap" routes P through bf16
    # copy/square stages first.  diag_paths: row-block i (mod 8) -> path.
    import ml_dtypes as _md
    paths = _cache.get("diag_paths", {})
    P64 = diag8 * (S1 * S1)                          # exact P_rr
    dcube = P64 ** 3 * C1                            # "dve" path value
    blk_i = (np.arange(V) // 128) % 8
    for i in range(8):
        if paths.get(i) == "apc":
            sel = blk_i == i
            p32 = P64[sel].astype(np.float32)
            s16 = p32.astype(_md.bfloat16).astype(np.float32)
            q16 = (p32 * p32).astype(_md.bfloat16).astype(np.float32)
            dcube[sel] = ((s16 * np.float32(C1)) * q16).astype(np.float64)
    G = np.ascontiguousarray(
        t8.T.reshape(3, 2, 128, V).transpose(2, 0, 1, 3))   # [128,3,2,V]

    # refinement operands: fp8 hi+lo (scaled x64)
    sens_idx = np.argsort(np.abs(den))[:REFINE_K]
    hi = (tn32 * S2).astype(F8)                      # [V, D]
    lo = (tn32 * S2 - hi.astype(np.float32)).astype(F8)
    H = np.ascontiguousarray(hi.T.reshape(3, 2, 128, V).transpose(2, 0, 1, 3))
    L = np.ascontiguousarray(lo.T.reshape(3, 2, 128, V).transpose(2, 0, 1, 3))
    h64 = hi.astype(np.float64)[sens_idx] / S2       # [K, D]
    l64 = lo.astype(np.float64)[sens_idx] / S2
    diag_ref = (h64 * h64 + 2 * h64 * l64).sum(1)    # device refined diagonal

    sens_hi = np.ascontiguousarray(H[:, :, :, sens_idx])
    sens_lo = np.ascontiguousarray(L[:, :, :, sens_idx])

    in_maps = []
    for c in range(NCORES):
        lc = (1024 * c + 512 + np.arange(LOCALW - 512)) % V
        in_maps.append({
            "mov8": np.ascontiguousarray(G[:, :, :, lc]),
            "sta8": np.ascontiguousarray(G[:, :, :, 1024 * c:1024 * (c + 1)]),
            "senshi": sens_hi,
            "senslo": sens_lo,
            "seghi": np.ascontiguousarray(H[:, :, :, 1024 * c:1024 * (c + 1)]),
            "seglo": np.ascontiguousarray(L[:, :, :, 1024 * c:1024 * (c + 1)]),
        })
    host = dict(den=den, collapse=collapse, dcube=dcube,
                sens_idx=sens_idx, diag_ref=diag_ref)
    return in_maps, host


def _assemble(results, host):
    den = host["den"]
    rowcube = np.zeros(V, dtype=np.float64)
    ref_acc = np.zeros(REFINE_K, dtype=np.float64)
    parts = np.arange(128)
    for c in range(NCORES):
        rc = results[c]["rc"].astype(np.float64)     # [128, 76]
        rc_main = rc[:, :RB_PER_CORE * NG].reshape(128, NG, RB_PER_CORE)
        # slot index = g*8 + i  (loop order: g outer, i inner)
        rows = rc_main.sum(axis=1)                   # [128, 8] (sum over g)
        rowcube[1024 * c:1024 * (c + 1)] += rows.T.reshape(-1)
        cols = results[c]["cols"].astype(np.float64)  # [128, NCB]
        for b in range(NCB):
            gcol = (1024 * c + 128 * b + parts) % V
            np.add.at(rowcube, gcol, cols[:, b])
        rcref = rc[:, RB_PER_CORE * NG:].reshape(128, RB, 2).sum(axis=2)
        ref_acc += rcref.T.reshape(-1)               # [256]
    rowcube -= host["dcube"]
    rowcube /= CSC
    ref_rows = ref_acc / CSC - host["diag_ref"] ** 3
    rowcube[host["sens_idx"]] = ref_rows
    hns = np.sum(rowcube / den)
    return np.float32(host["collapse"] + LAMBDA * hns)


def _get_runner():
    """Build + compile the Bass module once; wrap in a sharded-jit callable."""
    if "runner" in _cache:
        return _cache["runner"]

    import jax
    from jax.sharding import Mesh, PartitionSpec
    from jax.experimental.shard_map import shard_map
    from concourse import bass2jax, mybir

    nc = _build()
    bass2jax.install_neuronx_cc_hook()

    partition_name = (nc.partition_id_tensor.name
                      if nc.partition_id_tensor else None)
    in_names, out_names, out_avals, zero_outs = [], [], [], []
    for alloc in nc.m.functions[0].allocations:
        if not isinstance(alloc, mybir.MemoryLocationSet):
            continue
        name = alloc.memorylocations[0].name
        if alloc.kind == "ExternalInput":
            if name != partition_name:
                in_names.append(name)
        elif alloc.kind == "ExternalOutput":
            shape = tuple(alloc.tensor_shape)
            dtype = mybir.dt.np(alloc.dtype)
            out_names.append(name)
            out_avals.append(jax.core.ShapedArray(shape, dtype))
            zero_outs.append(np.zeros(shape, dtype))
    n_params = len(in_names)
    all_names = in_names + out_names
    if partition_name is not None:
        all_names = all_names + [partition_name]

    def _body(*args):
        operands = list(args)
        if partition_name is not None:
            operands.append(bass2jax.partition_id_tensor())
        outs = bass2jax._bass_exec_p.bind(
            *operands,
            out_avals=tuple(out_avals),
            in_names=tuple(all_names),
            out_names=tuple(out_names),
            lowering_input_output_aliases=(),
            sim_require_finite=True,
            sim_require_nnan=True,
            nc=nc,
        )
        return tuple(outs)

    devices = jax.devices()[:NCORES]
    mesh = Mesh(np.asarray(devices), ("core",))
    n_outs = len(out_names)
    sharded = jax.jit(
        shard_map(_body, mesh=mesh,
                  in_specs=(PartitionSpec("core"),) * (n_params + n_outs),
                  out_specs=(PartitionSpec("core"),) * n_outs,
                  check_rep=False),
        donate_argnums=tuple(range(n_params, n_params + n_outs)),
        keep_unused=True,
    )

    def execute(in_maps, device_inputs=None):
        if device_inputs is None:
            device_inputs = [
                np.concatenate([in_maps[c][nm] for c in range(NCORES)], axis=0)
                for nm in in_names
            ]
        concat_zeros = [
            np.zeros((NCORES * z.shape[0], *z.shape[1:]), z.dtype)
            for z in zero_outs
        ]
        out_arrs = sharded(*device_inputs, *concat_zeros)
        out_arrs = [np.asarray(a) for a in out_arrs]
        return [
            {nm: out_arrs[i].reshape(NCORES, *out_avals[i].shape)[c]
             for i, nm in enumerate(out_names)}
            for c in range(NCORES)
        ]

    runner = dict(nc=nc, execute=execute, in_names=in_names,
                  out_names=out_names, sharded=sharded, zero_outs=zero_outs,
                  out_avals=out_avals, mesh=mesh)
    _cache["runner"] = runner
    return runner


def _run(t_prime: np.ndarray):
    runner = _get_runner()
    in_maps, host = _prep(np.asarray(t_prime))
    results = runner["execute"](in_maps)
    loss = _assemble(results, host)
    return loss, results


def kernel(t_prime: np.ndarray) -> np.ndarray:
    loss, _ = _run(t_prime)
    return np.asarray(loss, dtype=np.float32)


def benchmark(t_prime: np.ndarray, iters: int = 20):
    """Repeat-execute with device-resident inputs; returns per-call seconds."""
    import time
    import jax
    runner = _get_runner()
    in_maps, host = _prep(np.asarray(t_prime))
    concat = [
        np.concatenate([in_maps[c][nm] for c in range(NCORES)], axis=0)
        for nm in runner["in_names"]
    ]
    from jax.sharding import NamedSharding, PartitionSpec
    sh = NamedSharding(runner["mesh"], PartitionSpec("core"))
    dev_in = [jax.device_put(a, sh) for a in concat]
    for a in dev_in:
        a.block_until_ready()
    runner["execute"](in_maps, device_inputs=dev_in)
    times = []
    for _ in range(iters):
        t0 = time.perf_counter()
        runner["execute"](in_maps, device_inputs=dev_in)
        times.append(time.perf_counter() - t0)
    return times



# revision 6
# speedup vs baseline: 1.1520x; 1.1520x over previous
"""TRN2 Bass kernel for nn_MFILoss_38225208934871 (wrap-symmetric fp8), v2.

loss = sum((diag(S)-1)^2) + 0.2 * sum_i [ sum_j S_off[i,j]^3 / (mean_j S_off[i,j] + 1e-6) ]
with S = t_norm @ t_norm.T, t_norm = L2-row-normalized t_prime [8192, 768].

Device strategy (8 cores, SPMD-identical program, host shards/gathers):
  - Wrap-symmetric decomposition: 64 row-blocks of 128; block i computes
    tiles (i, j) for j = i..i+32 (mod 64).  Every unordered pair d=1..31
    is computed once: the owner row-block row-reduces the cube tile, the
    partner gets it via a column-reduction (ones-matmul into PSUM).
    d = 32 tiles are computed at both ends (row-reductions only);
    diagonal blocks are row-reduced only.
  - Each core owns 8 consecutive row-blocks; its operand is one
    host-rotated column window [1024c, 1024c + 5120) mod 8192 (`loc8`),
    so all cores run the identical instruction stream on rotated data.
  - GEMMs in fp8e4m3 (inputs scaled x64 = S1) with DoubleRow perf mode
    (K=256 per matmul).
  - Cube+rowsum pipelines, chosen per tile by a greedy multi-engine
    balancer:
      "dve"  : fused custom-DVE cube+row-reduce straight from PSUM.
      "a16"  : ACT stages P->bf16, fused custom-DVE cube+reduce on bf16
               (SBUF operands; runs in the DVE 2x/4x perf mode that the
               engine reaches for 16-bit packed SBUF operands when the
               op's perf_max slot allows it).
      "apc"  : ACT stage+square, Pool multiply, ACT copy-with-accum.
      "pd_p"/"pd_a": ACT stage (+ACT or Pool square), DVE tensor-tensor
               cube; rowsum via batched DMA-engine transpose of the cube
               tile + ones-matmuls into a PSUM accumulator (PE), freeing
               DVE/ACT of the reduction entirely.
  - mean_neg is ill-conditioned; computed exactly on host (fp64).  The
    REFINE_K rows with smallest |mean_neg+eps| are recomputed on-device
    in fp8 hi+lo (~bf16 quality), column-sharded across cores.  hi == the
    bulk operand (same x64 scale), so only the lo residual is shipped.
  - Host assembles in fp64 (diag^3 subtraction, sensitive-row swap,
    final divide).

Inputs are full/unsharded; output is the full scalar loss (float32).
"""

import numpy as np
from contextlib import ExitStack

import os
V = 8192
D = 768
NCORES = 8
B = 128
RB_PER_CORE = 8              # row-blocks per core (1024 rows)
T = 33                       # staircase blocks per row-block (d = 0..32)
LOCALW = B * (RB_PER_CORE - 1) + B * T      # 5120 local columns
NCB = LOCALW // B            # 40 local column blocks (colsum slots)
NG = 9                       # 8 x 512-wide groups + 1 x 128-wide (d=32)

S1 = 64.0                    # fp8 input scale (bulk == refinement hi)
CSC = 4096.0                 # cube output scale: cb = S^3 * CSC
C1 = CSC / S1**6             # = 2^-24, exact
SCALE3 = float(C1) ** (1.0 / 3.0)            # = 2^-8, exact

REFINE_K = int(os.environ.get("K_REFINE", "128"))
RB = REFINE_K // 128         # stationary blocks of refined rows
SEGW = V // NCORES           # 1024-column refinement segment per core

EPS = 1e-6
LAMBDA = 0.2

# custom-DVE perf-mode ceiling declared on the fused cube op (byte-36
# perf_max).  The cost model grants the 2x/4x DVE rate only when the
# operands qualify (16-bit packed, SBUF) -- i.e. only on the "a16" path.
PERF_MAX = int(os.environ.get("K_PERF_MAX", "3"))
DMAT_BATCH = 4               # cube tiles per DMA-transpose instruction
N_WARMUP = int(os.environ.get("K_WARMUP", "24"))
EW_LAG_N = int(os.environ.get("K_EW_LAG", "3"))
PS_D = int(os.environ.get("K_PS_D", "2"))     # psum ring: dve lane
PS_A = int(os.environ.get("K_PS_A", "2"))     # psum ring: a16 lane
PS_X = int(os.environ.get("K_PS_X", "1"))     # psum ring: apc/pd/refine
DMA_BUDGET_NS = float(os.environ.get("K_DMA_BUDGET", "26000"))

# build-time engine cost estimates (ns) for the greedy balancer, per
# 512-wide tile.
_PIPE_COST = {
    "dve":  {"dve": 658.0},
    "a16":  {"act": float(os.environ.get("K_A16ACT", "612")), "dve": 194.0 if PERF_MAX >= 3 else
             (327.0 if PERF_MAX >= 1 else 593.0)},
    "apc":  {"act": 2023.0, "pool": 1111.0},
    "pd_p": {"act": 612.0, "pool": 1111.0, "dve": 327.0, "dma": 448.0},
    "pd_a": {"act": 1224.0, "dve": 327.0, "dma": 448.0},
}
# per-PAIR costs (two same-row 512-wide tiles fused into one [128,1024]
# PSUM region spanning 2 banks -> single ACT stage + single DVE cube)
_PAIR_COST = {
    "a2": {"act": 1038.0, "dve": 327.0 if PERF_MAX >= 3 else
           (593.0 if PERF_MAX >= 1 else 1126.0)},
    "ds": {"dve": 1316.0},
    "ac": {"act": 4046.0, "pool": 2222.0},
}
_INPUT_DMA_NS = 13300.0      # greedy balancer: dma clock head start

_cache = {}


def _register_cube_op():
    """Register a fused cube+row-reduce custom DVE op:
        out = (in0^2 * in0) * c1 ;  accum_out = c0 + sum(out_fp32)
    Follows the documented extension flow in concourse/dve_ops.py."""
    if "cube_op" in _cache:
        return _cache["cube_op"]
    import concourse.dve_ops as dve_ops
    from concourse.dve_ops import DveOp
    from concourse.dve_spec import Spec, Src0, C0, C1, sq, lower, _has_src1
    from concourse.dve_uop import DveOpSpec
    from operator import add
    import numpy as np_

    name = "CUBE_REDUCE_MFI"

    def _ref(in0, in1, c0, c1, c2):
        b = ((in0.astype(np_.float32) ** 2) * in0.astype(np_.float32)
             * c1).astype(np_.float32)
        return b, c0 + b.reshape(b.shape[0], -1).sum(axis=-1, keepdims=True)

    spec = Spec(body=sq(Src0) * Src0 * C1, accum=add, accum_init=C0,
                reference=_ref)

    if name not in dve_ops._SUB_OPCODE_FOR_NAME:
        row = max(dve_ops._SUB_OPCODE_FOR_NAME.values()) + 1
        assert row < 0x20
        dve_ops._SUB_OPCODE_FOR_NAME[name] = row

    shas = {}
    for ver in ("v3", "v4"):
        try:
            s = DveOpSpec(name=name,
                          opcode=dve_ops._SUB_OPCODE_FOR_NAME[name],
                          uops=lower(spec, ver=ver),
                          rd1_en=_has_src1(spec))
            shas[ver] = s.sha(ver)
        except Exception:
            pass
    op = DveOp(name, spec, subdim=False, uops_sha=shas)
    if all(o.name != name for o in dve_ops.OPS):
        dve_ops.OPS.append(op)
    dve_ops.CUSTOM_DVE_SPECS[name] = spec
    _cache["cube_op"] = op
    return op


def _build():
    import concourse.tile as tile
    from concourse import bacc, mybir

    F32 = mybir.dt.float32
    BF16 = mybir.dt.bfloat16
    FP8 = mybir.dt.float8e4
    MULT = mybir.AluOpType.mult
    DR = mybir.MatmulPerfMode.DoubleRow
    SQUARE = mybir.ActivationFunctionType.Square
    COPY = mybir.ActivationFunctionType.Copy

    nc = bacc.Bacc("TRN2", target_bir_lowering=False, debug=False,
                   num_devices=NCORES)

    d_loc = nc.dram_tensor("loc8", [128, 3, 2, LOCALW], FP8,
                           kind="ExternalInput").ap()
    d_shi = nc.dram_tensor("senshi", [128, 6 * REFINE_K], FP8,
                           kind="ExternalInput").ap()
    d_slo = nc.dram_tensor("senslo", [128, 6 * REFINE_K], FP8,
                           kind="ExternalInput").ap()
    d_glo = nc.dram_tensor("seglo", [128, 3, 2, SEGW], FP8,
                           kind="ExternalInput").ap()
    NREFT = RB * (SEGW // 512)
    NSLOT = RB_PER_CORE * NG + NREFT
    d_rc = nc.dram_tensor("rc", [128, NSLOT], F32,
                          kind="ExternalOutput").ap()
    # colsums [*, :NCB] and DMA-transpose rowsums [*, NCB:NCB+8]
    d_cols = nc.dram_tensor("cols", [128, NCB + RB_PER_CORE], F32,
                            kind="ExternalOutput").ap()

    cube_op = _register_cube_op()

    # greedy engine balancer state.  dma is tracked as a side constraint
    # (input streaming + pd transposes share the DMA engines), not in the
    # min-max objective.
    eng_t = {"act": 0.0, "dve": 0.0, "pool": 0.0}
    dma_t = [_INPUT_DMA_NS]
    DMA_BUDGET = DMA_BUDGET_NS

    def pick(width, allowed, table=None):
        scale = width / 512.0
        best = None
        for pname in allowed:
            costs = (table or _PIPE_COST)[pname]
            if "dma" in costs and dma_t[0] + costs["dma"] * scale > DMA_BUDGET:
                continue
            t = dict(eng_t)
            for e, c in costs.items():
                if e != "dma":
                    t[e] += c * scale
            m = max(t.values())
            if best is None or m < best[0]:
                best = (m, pname)
        pname = best[1]
        for e, c in (table or _PIPE_COST)[pname].items():
            if e == "dma":
                dma_t[0] += c * scale
            else:
                eng_t[e] += c * scale
        return pname

    with tile.TileContext(nc) as tc, ExitStack() as ctx:
        in_pool = ctx.enter_context(tc.tile_pool(name="inp", bufs=1))
        ref_pool = ctx.enter_context(tc.tile_pool(name="refin", bufs=1))
        psum_pool = ctx.enter_context(tc.tile_pool(name="ps", bufs=PS_A,
                                                   space="PSUM"))
        psum_d = ctx.enter_context(tc.tile_pool(name="psd", bufs=PS_D,
                                                space="PSUM"))
        psum_a = ctx.enter_context(tc.tile_pool(name="psa", bufs=PS_X,
                                                space="PSUM"))
        col_pool = ctx.enter_context(tc.tile_pool(name="cs", bufs=1,
                                                  space="PSUM"))
        sq_pool = ctx.enter_context(tc.tile_pool(name="sq", bufs=int(os.environ.get("K_SQBUFS", "16"))))
        sp_pool = ctx.enter_context(tc.tile_pool(
            name="sp16", bufs=int(os.environ.get("K_SPBUFS", "6"))))
        cbt_pool = ctx.enter_context(tc.tile_pool(name="cbt", bufs=2))
        cb_pool = ctx.enter_context(tc.tile_pool(name="cb", bufs=1))
        rc_pool = ctx.enter_context(tc.tile_pool(name="rc", bufs=1))

        # ---- inputs: one 5120-col window, streamed in pieces ----
        loc8 = in_pool.tile([128, 3, 2, LOCALW], FP8, tag="loc8")
        pieces = [(0, 512), (512, 1024), (1024, 1536), (1536, 2048), (2048, 3072)]
        for a, b in pieces:
            nc.sync.dma_start(loc8[:, :, :, a:b], d_loc[:, :, :, a:b])

        shi = ref_pool.tile([128, 3, 2, REFINE_K], FP8, tag="shi")
        nc.sync.dma_start(shi[:], d_shi)
        slo = ref_pool.tile([128, 3, 2, REFINE_K], FP8, tag="slo")
        nc.sync.dma_start(slo[:], d_slo)
        glo = ref_pool.tile([128, 3, 2, SEGW], FP8, tag="glo")
        nc.sync.dma_start(glo[:], d_glo)

        for a, b in [(3072, 4096), (4096, 5120)]:
            nc.sync.dma_start(loc8[:, :, :, a:b], d_loc[:, :, :, a:b])

        # ---- constants / buffers ----
        ones16 = cb_pool.tile([128, 1], BF16, tag="ones16")
        nc.gpsimd.memset(ones16[:], 1.0)
        zeros8 = cb_pool.tile([128, 128], FP8, tag="zeros8")
        nc.gpsimd.memset(zeros8[:], 0.0)
        NACC = NCB + RB_PER_CORE                 # 40 colsums + 8 rowsums
        onesNB = cb_pool.tile([128, NACC], FP8, tag="onesNB")
        nc.gpsimd.memset(onesNB[:], 1.0)
        # cube-tile arenas sized for the whole kernel: no reuse deps.
        # pd-path tiles go to cbD (consecutive slots per transpose batch).
        NB16 = NG * RB_PER_CORE + NREFT
        cbB = cb_pool.tile([128, NB16, 512], BF16, tag="cbB")

        rc = rc_pool.tile([128, NSLOT], F32, tag="rc")
        nc.gpsimd.memset(rc[:], 0.0)
        colsb = rc_pool.tile([128, NACC], F32, tag="colsb")

        # accumulation region: cols [0,NCB) = colsums, [NCB,NCB+8) = pd rowsums
        creg = col_pool.tile([128, NACC], F32, tag="creg")
        nc.tensor.matmul(creg[:], zeros8[:], onesNB[:],
                         start=True, stop=False, skip_group_check=True)

        # delayed colsum matmuls: (cb_ap_512wide, [(k, b)])
        pending = []
        ringb_i = [0]

        def flush_pending(keep=0):
            while len(pending) > keep:
                cb_ap, blocks = pending.pop(0)
                for (k, b) in blocks:
                    lhsT = cb_ap[:, 128 * k:128 * (k + 1)]
                    nc.tensor.matmul(creg[:, b:b + 1], lhsT, ones16[:],
                                     start=False, stop=False,
                                     skip_group_check=True)

        # pd-path rowsum machinery: batched DMA transpose + ones-matmuls
        pd_wait = []          # (nd_slot, i) not yet transposed
        pd_i = [0]            # next cbD slot

        def flush_pd(force=False):
            while len(pd_wait) >= DMAT_BATCH or (force and pd_wait):
                batch = pd_wait[:DMAT_BATCH]
                del pd_wait[:DMAT_BATCH]
                k = len(batch)
                nd0 = batch[0][0]
                cbT = cbt_pool.tile([128, 4 * DMAT_BATCH, 128], BF16,
                                    tag="cbT")
                nc.sync.dma_start_transpose(
                    cbT[:, :4 * k, :], cbD[:, nd0:nd0 + k, :])
                for t, (nd, i) in enumerate(batch):
                    assert nd == nd0 + t
                    for cix in range(4):
                        nc.tensor.matmul(
                            creg[:, NCB + i:NCB + i + 1],
                            cbT[:, 4 * t + cix, :], ones16[:],
                            start=False, stop=False, skip_group_check=True)

        # ---- PE warmup: keep the PE array busy (and its clock warm) while
        # the input DMAs stream in.
        WARM_N = int(os.environ.get("K_WARMN", "128"))
        for _ in range(N_WARMUP):
            Pd = psum_d.tile([128, 512], F32, tag="P")
            nc.tensor.matmul(Pd[:, :WARM_N], zeros8[:], zeros8[:, :WARM_N],
                             start=True, stop=True)

        def emit_elementwise(P, w, cb_out, slot, i, pname):
            if pname == "dve":
                r = nc.vector._custom_dve(cube_op, out=cb_out, in0=P[:, :w],
                                          s0=0.0, s1=C1, accum_out=slot)
                r.perf_max = PERF_MAX
            elif pname == "a16":
                s16 = sq_pool.tile([128, 512], BF16, tag="s16")
                nc.scalar.mul(s16[:, :w], P[:, :w], SCALE3)
                r = nc.vector._custom_dve(cube_op, out=cb_out,
                                          in0=s16[:, :w],
                                          s0=0.0, s1=1.0, accum_out=slot)
                r.perf_max = PERF_MAX
            elif pname == "apc":
                s16 = sq_pool.tile([128, 512], BF16, tag="s16")
                nc.scalar.mul(s16[:, :w], P[:, :w], SCALE3)
                sq = sq_pool.tile([128, 512], BF16, tag="sq")
                nc.scalar.activation(sq[:, :w], P[:, :w], SQUARE,
                                     scale=SCALE3)
                ct = sq_pool.tile([128, 512], BF16, tag="ct")
                nc.gpsimd.tensor_tensor(ct[:, :w], s16[:, :w], sq[:, :w],
                                        MULT)
                nc.scalar.activation(cb_out, ct[:, :w], COPY,
                                     accum_out=slot)
            else:  # pd_p / pd_a: rowsum via transpose; no slot written
                s16 = sq_pool.tile([128, 512], BF16, tag="s16")
                nc.scalar.mul(s16[:, :w], P[:, :w], SCALE3)
                sq = sq_pool.tile([128, 512], BF16, tag="sq")
                if pname == "pd_p":
                    nc.gpsimd.tensor_tensor(sq[:, :w], s16[:, :w],
                                            s16[:, :w], MULT)
                else:
                    nc.scalar.activation(sq[:, :w], P[:, :w], SQUARE,
                                         scale=SCALE3)
                nc.vector.tensor_tensor(cb_out, sq[:, :w], s16[:, :w], MULT)

        ref_tiles = [(rb, qq) for rb in range(RB)
                     for qq in range(SEGW // 512)]

        def emit_refinement_tile(rb, qq):
            # refinement rhs: hi = loc8's own first 1024 cols, lo = glo
            P = psum_a.tile([128, 512], F32, tag="P")
            ref_ops = [(shi, loc8), (shi, glo), (slo, loc8)]
            n_mm = len(ref_ops) * 3
            i_mm = 0
            for (wl, wr) in ref_ops:
                for kc2 in range(3):
                    nc.tensor.matmul(
                        P[:],
                        wl[:, kc2, :, rb * 128:(rb + 1) * 128],
                        wr[:, kc2, :, qq * 512:(qq + 1) * 512],
                        start=(i_mm == 0), stop=(i_mm == n_mm - 1),
                        perf_mode=DR)
                    i_mm += 1
            nb = ringb_i[0]
            ringb_i[0] += 1
            slot = rc[:, RB_PER_CORE * NG + rb * 2 + qq:
                      RB_PER_CORE * NG + rb * 2 + qq + 1]
            # exact-precision pipeline: the refinement rows are den-sensitive
            sqf = sq_pool.tile([128, 512], F32, tag="sqf")
            nc.scalar.activation(sqf[:], P[:], SQUARE)
            nc.vector.scalar_tensor_tensor(cbB[:, nb, :], P[:], C1, sqf[:],
                                           MULT, MULT, accum_out=slot)
            eng_t["act"] += 612.0
            eng_t["dve"] += 658.0

        def emit_pair(g0, i):
            """Two same-row tiles (g0, g0+1) as one unit.  "a2" runs
            them through one [128,1024] 2-bank PSUM region with a single
            ACT stage + single fused DVE cube (their rowsums share one
            slot -- the host sums slots per row anyway); "ds"/"ac" fall
            back to two independent single-tile pipelines."""
            pname = pick(1024, PAIR_PATHS, table=_PAIR_COST)
            if g0 == 0:
                _cache.setdefault("diag_paths", {})[i] = {
                    "a2": "a16", "ds": "dve", "ac": "apc"}[pname]

            if pname == "a2":
                P2 = psum_pool.tile([128, 1024], F32, tag="P2")
                for h in range(2):
                    c0 = 128 * i + 512 * (g0 + h)
                    for kc2 in range(3):
                        nc.tensor.matmul(
                            P2[:, 512 * h:512 * h + 512],
                            loc8[:, kc2, :, 128 * i:128 * (i + 1)],
                            loc8[:, kc2, :, c0:c0 + 512],
                            start=(kc2 == 0), stop=(kc2 == 2),
                            perf_mode=DR, skip_group_check=True)
                nb = ringb_i[0]
                ringb_i[0] += 2
                blocks = _pair_blocks(g0, i)
                ew_queue.append(("a2", P2, nb, i, g0, blocks))
            else:
                single = {"ds": "dve", "ac": "apc"}[pname]
                for h in range(2):
                    _emit_single(g0 + h, i, single)
            drain_ew(keep=EW_LAG)

        def _pair_blocks(g0, i):
            blocks = []
            for h in range(2):
                g = g0 + h
                b0 = i + 4 * g
                ks = range(1, 4) if g == 0 else range(4)
                blocks.append([(k, b0 + k) for k in ks])
            return blocks

        def _emit_single(g, i, pname):
            w = 512 if g < NG - 1 else 128
            c0 = 128 * i + 512 * g
            P = (psum_d if pname == "dve" else psum_a).tile(
                [128, 512], F32, tag="P")
            for kc2 in range(3):
                nc.tensor.matmul(
                    P[:, :w],
                    loc8[:, kc2, :, 128 * i:128 * (i + 1)],
                    loc8[:, kc2, :, c0:c0 + w],
                    start=(kc2 == 0), stop=(kc2 == 2), perf_mode=DR)
            nb = ringb_i[0]
            ringb_i[0] += 1
            b0 = i + 4 * g
            if g == 0:
                _cache.setdefault("diag_paths", {})[i] = pname
                blocks = [(k, b0 + k) for k in range(1, 4)]
            elif g < NG - 1:
                blocks = [(k, b0 + k) for k in range(4)]
            else:
                blocks = []
            ew_queue.append(("s", P, w, nb, g * RB_PER_CORE + i, pname,
                             blocks))

        def emit_narrow(i):
            pname = pick(128, ("dve", "apc"))
            _emit_single(NG - 1, i, pname)

        # software-pipelined consumers
        ew_queue = []
        EW_LAG = EW_LAG_N

        def drain_ew(keep=0):
            while len(ew_queue) > keep:
                ent = ew_queue.pop(0)
                if ent[0] == "a2":
                    _, P2, nb, i, g0, blocks = ent
                    slot = rc[:, g0 * RB_PER_CORE + i:
                              g0 * RB_PER_CORE + i + 1]
                    s16 = sp_pool.tile([128, 1024], BF16, tag="s16p")
                    nc.scalar.mul(s16[:], P2[:], SCALE3)
                    r = nc.vector._custom_dve(
                        cube_op, out=cbB[:, nb:nb + 2, :],
                        in0=s16[:], s0=0.0, s1=1.0, accum_out=slot)
                    r.perf_max = PERF_MAX
                    for h in range(2):
                        if blocks[h]:
                            pending.append((cbB[:, nb + h, :], blocks[h]))
                else:
                    _, P, w, nb, tid, pname, blocks = ent
                    slot = rc[:, tid:tid + 1]
                    emit_elementwise(P, w, cbB[:, nb, :w], slot, 0, pname)
                    if blocks:
                        pending.append((cbB[:, nb, :], blocks))
                flush_pending(keep=int(os.environ.get("K_KEEP", "12")))

        PAIR_PATHS = tuple(os.environ.get(
            "K_PAIRS", "a2,ds,ac").split(","))

        for gp in range(4):
            for i in range(RB_PER_CORE):
                emit_pair(2 * gp, i)
                if gp == 3:
                    emit_narrow(i)
                if gp in (2, 3) and i == 3 and ref_tiles:
                    drain_ew()
                    emit_refinement_tile(*ref_tiles.pop(0))
                if gp == 3 and i == 6 and ref_tiles:
                    drain_ew()
                    emit_refinement_tile(*ref_tiles.pop(0))

        drain_ew()
        while ref_tiles:
            emit_refinement_tile(*ref_tiles.pop(0))
        flush_pending()

        # ---- outputs ----
        # colsums/pd-rowsums and main slots complete before the refinement
        # tail ends; ship them early so only the 4 refinement slots drain
        # at the end.
        nc.sync.dma_start(d_rc[:, :RB_PER_CORE * NG],
                          rc[:, :RB_PER_CORE * NG])
        nc.scalar.copy(colsb[:], creg[:])
        nc.sync.dma_start(d_cols, colsb[:])
        nc.sync.dma_start(d_rc[:, RB_PER_CORE * NG:],
                          rc[:, RB_PER_CORE * NG:])

    nc.compile()

    # Declare the perf-mode ceiling on the fused cube ops (byte-36
    # perf_max; the engine reaches slot <= perf_max at runtime).  The
    # cost model grants 2x/4x only where operand dtypes/spaces qualify.
    if PERF_MAX:
        for blk in nc.m.functions[0].blocks:
            for inst in blk.instructions:
                if type(inst).__name__ == "InstCustomDveAnt":
                    inst.perf_max = PERF_MAX
    return nc


def _prep(t_prime: np.ndarray):
    import ml_dtypes
    F8 = ml_dtypes.float8_e4m3fn

    t64 = t_prime.astype(np.float64)
    norm = np.maximum(np.sqrt((t64 * t64).sum(1, keepdims=True)), 1e-12)
    tn32 = (t64 / norm).astype(np.float32)           # [V, D]
    tn64 = tn32.astype(np.float64)

    # exact (fp64) mean_neg and collapse on host
    s = tn64.sum(0)
    rowsum = tn64 @ s
    diag = (tn64 * tn64).sum(1)
    mean_neg = (rowsum - diag) / (V - 1)
    den = mean_neg + EPS
    collapse = np.sum((diag - 1.0) ** 2)

    # fp8 bulk operand (scaled x64), in device layout [128, 3, 2, V]
    t8 = (tn32 * S1).astype(F8)                      # [V, D] fp8 == hi
    t8_64 = t8.astype(np.float64) / S1
    diag8 = (t8_64 * t8_64).sum(1)                   # device bulk diagonal
    # device diagonal cube (pre-/CSC scale), path-dependent per row-block
    # (mod 8): "dve" cubes the exact fp32 P; "a16"/"apc" route P through
    # bf16 stages first.  diag_paths is recorded at build time.
    bf = ml_dtypes.bfloat16
    P64 = diag8 * (S1 * S1)
    dcube = P64 ** 3 * C1                            # "dve" path value
    paths = _cache.get("diag_paths", {})
    blk_i = (np.arange(V) // 128) % 8
    p32 = (P64 * SCALE3).astype(np.float32)          # P * 2^-8, fp32
    s16 = p32.astype(bf).astype(np.float64)          # bf16 stage
    for i in range(8):
        pth = paths.get(i, "dve")
        sel = blk_i == i
        if pth == "a16":
            dcube[sel] = (s16[sel] ** 3)
        elif pth == "apc":
            sq = (p32[sel] * p32[sel]).astype(bf).astype(np.float64)
            ct = (s16[sel] * sq).astype(np.float32).astype(bf)
            dcube[sel] = ct.astype(np.float64)
    G = np.ascontiguousarray(
        t8.T.reshape(3, 2, 128, V).transpose(2, 0, 1, 3))   # [128,3,2,V]

    # refinement lo residual (hi == t8)
    sens_idx = np.argsort(np.abs(den))[:REFINE_K]
    lo = (tn32 * S1 - t8.astype(np.float32)).astype(F8)
    L = np.ascontiguousarray(lo.T.reshape(3, 2, 128, V).transpose(2, 0, 1, 3))
    h64 = t8.astype(np.float64)[sens_idx] / S1       # [K, D]
    l64 = lo.astype(np.float64)[sens_idx] / S1
    diag_ref = (h64 * h64 + 2 * h64 * l64).sum(1)    # device refined diagonal

    sens_hi = np.ascontiguousarray(G[:, :, :, sens_idx]).reshape(128, -1)
    sens_lo = np.ascontiguousarray(L[:, :, :, sens_idx]).reshape(128, -1)

    in_maps = []
    for c in range(NCORES):
        lc = (1024 * c + np.arange(LOCALW)) % V
        in_maps.append({
            "loc8": np.ascontiguousarray(G[:, :, :, lc]),
            "senshi": sens_hi,
            "senslo": sens_lo,
            "seglo": np.ascontiguousarray(L[:, :, :, 1024 * c:1024 * (c + 1)]),
        })
    host = dict(den=den, collapse=collapse, dcube=dcube,
                sens_idx=sens_idx, diag_ref=diag_ref)
    return in_maps, host


def _assemble(results, host):
    den = host["den"]
    rowcube = np.zeros(V, dtype=np.float64)
    ref_acc = np.zeros(REFINE_K, dtype=np.float64)
    parts = np.arange(128)
    NREFT = RB * (SEGW // 512)
    for c in range(NCORES):
        rc = results[c]["rc"].astype(np.float64)     # [128, NSLOT]
        rc_main = rc[:, :RB_PER_CORE * NG].reshape(128, NG, RB_PER_CORE)
        rows = rc_main.sum(axis=1)                   # [128, 8] (sum over g)
        cols = results[c]["cols"].astype(np.float64)  # [128, NCB+8]
        # pd-path rowsums accumulated on-device
        rows = rows + cols[:, NCB:NCB + RB_PER_CORE]
        rowcube[1024 * c:1024 * (c + 1)] += rows.T.reshape(-1)
        for b in range(NCB):
            gcol = (1024 * c + 128 * b + parts) % V
            np.add.at(rowcube, gcol, cols[:, b])
        rcref = rc[:, RB_PER_CORE * NG:].reshape(128, RB, 2).sum(axis=2)
        ref_acc += rcref.T.reshape(-1)               # [REFINE_K]
    rowcube -= host["dcube"]
    rowcube /= CSC
    ref_rows = ref_acc / CSC - host["diag_ref"] ** 3
    rowcube[host["sens_idx"]] = ref_rows
    hns = np.sum(rowcube / den)
    return np.float32(host["collapse"] + LAMBDA * hns)


def _get_runner():
    """Build + compile the Bass module once; wrap in a sharded-jit callable."""
    if "runner" in _cache:
        return _cache["runner"]

    import jax
    from jax.sharding import Mesh, PartitionSpec
    from jax.experimental.shard_map import shard_map
    from concourse import bass2jax, mybir

    nc = _build()
    bass2jax.install_neuronx_cc_hook()

    partition_name = (nc.partition_id_tensor.name
                      if nc.partition_id_tensor else None)
    in_names, out_names, out_avals, zero_outs = [], [], [], []
    for alloc in nc.m.functions[0].allocations:
        if not isinstance(alloc, mybir.MemoryLocationSet):
            continue
        name = alloc.memorylocations[0].name
        if alloc.kind == "ExternalInput":
            if name != partition_name:
                in_names.append(name)
        elif alloc.kind == "ExternalOutput":
            shape = tuple(alloc.tensor_shape)
            dtype = mybir.dt.np(alloc.dtype)
            out_names.append(name)
            out_avals.append(jax.core.ShapedArray(shape, dtype))
            zero_outs.append(np.zeros(shape, dtype))
    n_params = len(in_names)
    all_names = in_names + out_names
    if partition_name is not None:
        all_names = all_names + [partition_name]

    def _body(*args):
        operands = list(args)
        if partition_name is not None:
            operands.append(bass2jax.partition_id_tensor())
        outs = bass2jax._bass_exec_p.bind(
            *operands,
            out_avals=tuple(out_avals),
            in_names=tuple(all_names),
            out_names=tuple(out_names),
            lowering_input_output_aliases=(),
            sim_require_finite=True,
            sim_require_nnan=True,
            nc=nc,
        )
        return tuple(outs)

    devices = jax.devices()[:NCORES]
    mesh = Mesh(np.asarray(devices), ("core",))
    n_outs = len(out_names)
    sharded = jax.jit(
        shard_map(_body, mesh=mesh,
                  in_specs=(PartitionSpec("core"),) * (n_params + n_outs),
                  out_specs=(PartitionSpec("core"),) * n_outs,
                  check_rep=False),
        donate_argnums=tuple(range(n_params, n_params + n_outs)),
        keep_unused=True,
    )

    def execute(in_maps, device_inputs=None):
        if device_inputs is None:
            device_inputs = [
                np.concatenate([in_maps[c][nm] for c in range(NCORES)], axis=0)
                for nm in in_names
            ]
        concat_zeros = [
            np.zeros((NCORES * z.shape[0], *z.shape[1:]), z.dtype)
            for z in zero_outs
        ]
        out_arrs = sharded(*device_inputs, *concat_zeros)
        out_arrs = [np.asarray(a) for a in out_arrs]
        return [
            {nm: out_arrs[i].reshape(NCORES, *out_avals[i].shape)[c]
             for i, nm in enumerate(out_names)}
            for c in range(NCORES)
        ]

    runner = dict(nc=nc, execute=execute, in_names=in_names,
                  out_names=out_names, sharded=sharded, zero_outs=zero_outs,
                  out_avals=out_avals, mesh=mesh)
    _cache["runner"] = runner
    return runner


def _run(t_prime: np.ndarray):
    runner = _get_runner()
    in_maps, host = _prep(np.asarray(t_prime))
    results = runner["execute"](in_maps)
    loss = _assemble(results, host)
    return loss, results


def kernel(t_prime: np.ndarray) -> np.ndarray:
    loss, _ = _run(t_prime)
    return np.asarray(loss, dtype=np.float32)


def benchmark(t_prime: np.ndarray, iters: int = 20):
    """Repeat-execute with device-resident inputs; returns per-call seconds."""
    import time
    import jax
    runner = _get_runner()
    in_maps, host = _prep(np.asarray(t_prime))
    concat = [
        np.concatenate([in_maps[c][nm] for c in range(NCORES)], axis=0)
        for nm in runner["in_names"]
    ]
    from jax.sharding import NamedSharding, PartitionSpec
    sh = NamedSharding(runner["mesh"], PartitionSpec("core"))
    dev_in = [jax.device_put(a, sh) for a in concat]
    for a in dev_in:
        a.block_until_ready()
    runner["execute"](in_maps, device_inputs=dev_in)
    times = []
    for _ in range(iters):
        t0 = time.perf_counter()
        runner["execute"](in_maps, device_inputs=dev_in)
        times.append(time.perf_counter() - t0)
    return times
